# revision 9
# baseline (speedup 1.0000x reference)
"""Trainium2 Bass kernel for the FSRS-like scan (T=256, B=32768).

Data-parallel over the batch: 8 cores x 4096 elements, the T=256 recurrence
runs locally per core with all ~30 scalar parameters baked into instruction
immediates (JIT-specialized per call).

Per step, per core, tiles are [128 partitions x 32 free].  The step function
is decomposed into ScalarE activations (Exp/Ln/Square from the
natural_log_exp_and_others table set - a single table load for the whole
kernel), fused VectorE custom ops (ln_bwd_dx / affine_then_add /
reciprocal_approx_fast), and GPSIMD tensor ops for off-critical-path work.

Key identities used (all equivalent to the reference up to fp rounding):
  mish(x)       = x - 2*x/(min((1+e^x)^2,1e35)+1)     (overflow-safe)
  sigmoid(g)    = 1/(1+e^(-g))          via reciprocal_approx
  softplus(y)   = ln(e^y + 1)           (+1 folded into Ln bias)
  rw = clip(exp(-softplus(y)))          == exp(-clip(softplus(y)))
  log(rw_clipped) == -softplus_clipped  (saves the Ln of rw)
"""

import numpy as np

T = 256
B = 32768
NCORES = 8
BC = B // NCORES   # 4096
P = 128
FD = BC // P       # 32

F32 = None  # set lazily to mybir.dt.float32


def _f(x):
    return float(np.float32(x))


class StepBuilder:
    """Emits the per-step op sequence. nc engines: A=scalar, D=vector, G=gpsimd."""

    def __init__(self, nc, tc, pools, params):
        self.nc = nc
        self.tc = tc
        self.pools = pools
        self.p = params
        import concourse.mybir as mybir
        self.mybir = mybir
        self.AF = mybir.ActivationFunctionType
        self.OP = mybir.AluOpType

    # ---- emission helpers -------------------------------------------------
    def tile(self, tag, fd=FD):
        return self.pools["wk"].tile([P, fd], self._dt(), tag=tag, name=tag)

    def _dt(self):
        return self.mybir.dt.float32

    def ts(self, eng, out, in0, s1, op0, s2=None, op1=None):
        kw = dict(out=out, in0=in0, scalar1=_f(s1),
                  scalar2=None if s2 is None else _f(s2), op0=op0)
        if op1 is not None:
            kw["op1"] = op1
        eng.tensor_scalar(**kw)

    def stt(self, eng, out, in0, s, in1, op0, op1):
        eng.scalar_tensor_tensor(out=out, in0=in0, scalar=_f(s), in1=in1,
                                 op0=op0, op1=op1)

    def tt(self, eng, out, in0, in1, op):
        eng.tensor_tensor(out=out, in0=in0, in1=in1, op=op)

    def act(self, out, in_, func, scale=1.0, bias=0.0):
        self.nc.scalar.activation(out, in_, func, bias=_f(bias), scale=_f(scale))

    def lnbwd(self, out, dy, x_hat, s0, s1, c2):
        self.nc.vector.ln_bwd_dx(out=out, dy=dy, x_hat=x_hat,
                                 mean_dyx=_f(s0), mean_dy=_f(s1), scale=_f(c2))

    def ata(self, out, in0, scale, bias, in1):
        self.nc.vector.affine_then_add(out=out, in0=in0, in1=in1,
                                       scale=_f(scale), bias=_f(bias))

    def recip(self, out, in_):
        self.nc.vector.reciprocal_approx_fast(out=out, in_=in_)

    def two_term(self, out, A_ap, a, B_ap, b, bias):
        """out = A*a + B*b + bias in one DVE op (anchor on larger |coef|)."""
        if abs(a) >= abs(b):
            assert abs(a) > 1e-30
            self.lnbwd(out, A_ap, B_ap, -b / a, -bias / a, a)
        else:
            assert abs(b) > 1e-30
            self.lnbwd(out, B_ap, A_ap, -a / b, -bias / b, b)

    def mish_tail(self, h, name, cA, cB, out_scale_sign):
        """From h, compute y = cA*(h - 2*t) + cB where t = h/(min((1+e^h)^2,1e35)+1).
        out_scale_sign=+1 gives y; -1 gives -y.  Returns the y tile.
        Ops: Exp[A], Square[A], ts[D/G], recip[D], tt[D/G], lnbwd[D]."""
        OP, AF = self.OP, self.AF
        e = self.tile(f"e_{name}")
        q = self.tile(f"q_{name}")
        v = self.tile(f"v_{name}")
        iv = self.tile(f"iv_{name}")
        t = self.tile(f"t_{name}")
        y = self.tile(f"y_{name}")
        self.act(e, h, AF.Exp)
        self.act(q, e, AF.Square, 1.0, 1.0)             # (e+1)^2
        self.ts(self.nc.gpsimd, v, q, 1e35, OP.min, 1.0, OP.add)
        self.recip(iv, v)
        self.tt(self.nc.gpsimd, t, h, iv, OP.mult)
        # y = cA*(h-2t)+cB = (t - h*0.5 - (-cB/(2cA))) * (-2cA); sign folds in.
        sgn = out_scale_sign
        assert abs(cA) > 1e-30
        self.lnbwd(y, t, h, 0.5, cB / (2 * cA), -2 * cA * sgn)
        return y


def build_bass(params, T_steps=None):
    import concourse.mybir as mybir
    from concourse.tile import TileContext

    f32 = mybir.dt.float32
    AF = mybir.ActivationFunctionType
    OP = mybir.AluOpType

    from concourse import bacc
    nc = bacc.Bacc()
    dt_d = nc.dram_tensor("dt", [T, P, FD], f32, kind="ExternalInput")
    rat_d = nc.dram_tensor("rat", [T, P, FD], f32, kind="ExternalInput")
    lap_d = nc.dram_tensor("lap", [T, P, FD], f32, kind="ExternalInput")
    out_d = nc.dram_tensor("out", [T, P, 2 * FD], f32, kind="ExternalOutput")

    p = params
    p0, p1, p2, p3, p4, p5 = p["rw_p"]
    q0, q1, q2, q3, q4, q5 = p["next_d_p"]
    r0, r1, r2, r3, r4 = p["pls_p"]
    c0, c1, c2, c3, c4, c5 = p["sinc_p"]
    b0, b1, b2, b3, b4 = p["best_sinc_p"]
    w0, w1, w2 = p["sinc_w"]
    S0 = p["S0"]
    D0 = p["D0"]
    Cb = b0 + b1 + b2
    LOG09 = -0.10536051565782628
    S_MIN, S_MAX = 0.01, 36500.0
    SPLO = _f(-np.log(np.float32(0.9999)))
    SPHI = _f(-np.log(np.float32(1e-4)))

    # Register const APs for non-{0,1} ACT bias values (bias lowers to a
    # [128,1] SBUF constant operand).
    def reg_const(val):
        v = _f(val)
        key = (f32, v)
        if key not in nc.const_aps.aps:
            tns = nc.alloc_sbuf_tensor(f"const-f32-{v}", [128, 1], f32)
            nc.gpsimd.memset(tns.ap(), v)
            nc.const_aps.aps[key] = tns.ap()

    reg_const(w0)
    nc.all_engine_barrier()

    with TileContext(nc) as tc:
        with tc.tile_pool(name="io", bufs=6) as io, \
             tc.tile_pool(name="st", bufs=3) as st, \
             tc.tile_pool(name="wk", bufs=2) as wk:
            pools = {"io": io, "st": st, "wk": wk}
            sb = StepBuilder(nc, tc, pools, p)
            A, D, G = nc.scalar, nc.vector, nc.gpsimd

            # ---------------- t = 0: init branch --------------------------
            rat0 = io.tile([P, FD], f32, tag="rat", name="rat0")
            nc.sync.dma_start(out=rat0, in_=rat_d[0])
            out0 = st.tile([P, 2 * FD], f32, tag="out", name="out")
            s_ap0 = out0[:, 0:FD]
            d_ap0 = out0[:, FD:]
            mk = [sb.tile(f"mask{k}") for k in range(4)]
            for k in range(4):
                sb.ts(D, mk[k], rat0, float(k + 1), OP.is_equal)
            acc_s = sb.tile("acc_s")
            acc_d = sb.tile("acc_d")
            sb.ts(D, acc_s, mk[0], S0[0], OP.mult)
            sb.ts(D, acc_d, mk[0], D0[0], OP.mult)
            for k in range(1, 4):
                tgt_s = s_ap0 if k == 3 else acc_s
                tgt_d = d_ap0 if k == 3 else acc_d
                sb.stt(D, tgt_s, mk[k], S0[k], acc_s, OP.mult, OP.add)
                sb.stt(D, tgt_d, mk[k], D0[k], acc_d, OP.mult, OP.add)
            nc.sync.dma_start(out=out_d[0], in_=out0)

            inv_s = sb.tile("inv_s")
            sb.recip(inv_s, s_ap0)
            prev_out = out0

            # ---------------- steps 1..T-1 ---------------------------------
            for t in range(1, T_steps if T_steps is not None else T):
                s_prev = prev_out[:, 0:FD]
                d_prev = prev_out[:, FD:]

                dt_t = io.tile([P, FD], f32, tag="dt", name="dt_t")
                rat_t = io.tile([P, FD], f32, tag="rat", name="rat_t")
                lap_t = io.tile([P, FD], f32, tag="lap", name="lap_t")
                nc.sync.dma_start(out=dt_t, in_=dt_d[t])
                nc.sync.dma_start(out=rat_t, in_=rat_d[t])
                nc.sync.dma_start(out=lap_t, in_=lap_d[t])

                out_t = st.tile([P, 2 * FD], f32, tag="out", name="out")
                s_ap = out_t[:, 0:FD]
                d_ap = out_t[:, FD:]

                # --- rt chain (critical start) ---
                arg = sb.tile("arg")
                sb.stt(D, arg, inv_s, LOG09, dt_t, OP.mult, OP.mult)
                rt = sb.tile("rt")
                sb.act(rt, arg, AF.Exp)
                rt_c = sb.tile("rt_c")
                sb.ts(D, rt_c, rt, 1e-4, OP.max, 0.9999, OP.min)

                # --- h1 = d*p0 + s*p1 + rt_c*p2 + p3 ---
                pre1 = sb.tile("pre1")
                sb.ts(G, pre1, d_prev, p0, OP.mult, p3, OP.add)
                x1 = sb.tile("x1")
                sb.ata(x1, s_prev, p1, 0.0, pre1)
                h1 = sb.tile("h1")
                sb.ata(h1, rt_c, p2, 0.0, x1)

                # --- mish1 -> y = p4*mish+p5 ---
                y = sb.mish_tail(h1, "m1", p4, p5, +1)

                # --- rw / softplus(y) ---
                ey = sb.tile("ey")
                sb.act(ey, y, AF.Exp)
                spy = sb.tile("spy")
                sb.act(spy, ey, AF.Ln, 1.0, 1.0)
                spy_c = sb.tile("spy_c")
                sb.ts(D, spy_c, spy, SPLO, OP.max, SPHI, OP.min)
                rw = sb.tile("rw")
                sb.act(rw, spy_c, AF.Exp, -1.0, 0.0)
                rsp = sb.tile("rsp")
                sb.recip(rsp, spy_c)
                sr_raw = sb.tile("sr_raw")
                sb.stt(D, sr_raw, rsp, -LOG09, dt_t, OP.mult, OP.mult)
                sr = sb.tile("sr")
                sb.ts(D, sr, sr_raw, S_MIN, OP.max, S_MAX, OP.min)

                # --- nd chain: h2 = d*q0 + rw*q1 + (rat*q2+q3) ---
                B_t = sb.tile("B_t")
                sb.ts(G, B_t, rat_t, q2, OP.mult, q3, OP.add)
                B2a = sb.tile("B2a")
                sb.ts(G, B2a, d_prev, q0, OP.mult)
                B2 = sb.tile("B2")
                sb.tt(G, B2, B2a, B_t, OP.add)
                h2 = sb.tile("h2")
                sb.ata(h2, rw, q1, 0.0, B2)
                negg = sb.mish_tail(h2, "m2", q4, q5, -1)
                eg = sb.tile("eg")
                sb.act(eg, negg, AF.Exp)
                ug = sb.tile("ug")
                sb.ts(G, ug, eg, 1.0, OP.add)
                sb.recip(d_ap, ug)  # nd written strided into out tile

                # --- pls chain: h3 = rw*r0 + (lap*r1+r2) ---
                C_t = sb.tile("C_t")
                sb.ts(G, C_t, lap_t, r1, OP.mult, r2, OP.add)
                h3 = sb.tile("h3")
                sb.ata(h3, rw, r0, 0.0, C_t)
                k_ = sb.mish_tail(h3, "m3", r3, r4, +1)
                ek = sb.tile("ek")
                sb.act(ek, k_, AF.Exp)
                pls = sb.tile("pls")
                sb.act(pls, ek, AF.Ln, 1.0, 1.0)
                pls_c = sb.tile("pls_c")
                sb.ts(G, pls_c, pls, S_MIN, OP.max, S_MAX, OP.min)
                minv = sb.tile("minv")
                sb.ts(G, minv, rat_t, 1.5, OP.is_le)
                pls0 = sb.tile("pls0")
                sb.tt(G, pls0, pls_c, minv, OP.mult)

                # --- sinc_t: ez ---
                ez = sb.tile("ez")
                if w1 == 0.0:
                    sb.act(ez, rw, AF.Exp, -w2, w0)
                else:
                    lnsr = sb.tile("lnsr")
                    sb.act(lnsr, sr, AF.Ln)
                    za = sb.tile("za")
                    sb.ts(D, za, lnsr, -w1, OP.mult, w0, OP.add)
                    ezarg = sb.tile("ezarg")
                    sb.ata(ezarg, rw, -w2, 0.0, za)
                    sb.act(ez, ezarg, AF.Exp)

                # --- sinc_nn: h4 = nd*c0 + sr*c1 + rw*c2 + c3 ---
                h4a = sb.tile("h4a")
                sb.two_term(h4a, d_ap, c0, sr, c1, 0.0)
                h4 = sb.tile("h4")
                sb.ata(h4, rw, c2, c3, h4a)
                q4v = sb.mish_tail(h4, "m4", c4, c5, +1)
                q4c = sb.tile("q4c")
                sb.ts(D, q4c, q4v, 80.0, OP.min)
                eq = sb.tile("eq")
                sb.act(eq, q4c, AF.Exp)
                vnn = sb.tile("vnn")
                sb.act(vnn, eq, AF.Ln, 1.0, 1.0)

                # --- best ---
                f_t = sb.tile("f_t")
                sb.ts(G, f_t, d_ap, -5.0, OP.mult, 6.0, OP.add)
                u_t = sb.tile("u_t")
                sb.tt(D, u_t, f_t, ez, OP.mult)
                h5 = sb.tile("h5")
                sb.two_term(h5, u_t, b0, vnn, b1, Cb)
                r5 = sb.mish_tail(h5, "m5", b3, b4, +1)
                r5c = sb.tile("r5c")
                sb.ts(D, r5c, r5, 80.0, OP.min)
                er = sb.tile("er")
                sb.act(er, r5c, AF.Exp)
                wb = sb.tile("wb")
                sb.act(wb, er, AF.Ln, 1.0, 1.0)

                # --- combine: s_rec = M*sr*(wb+1) + (1-M)*pls_c, clip ---
                sbt = sb.tile("sbt")
                sb.stt(D, sbt, wb, 1.0, sr, OP.add, OP.mult)
                m_t = sb.tile("m_t")
                sb.ts(G, m_t, rat_t, 1.5, OP.is_gt)
                sb_m = sb.tile("sb_m")
                sb.tt(D, sb_m, sbt, m_t, OP.mult)
                s_rec = sb.tile("s_rec")
                sb.tt(D, s_rec, sb_m, pls0, OP.add)
                sb.ts(D, s_ap, s_rec, S_MIN, OP.max, S_MAX, OP.min)

                nc.sync.dma_start(out=out_d[t], in_=out_t)

                inv_s = sb.tile("inv_s")
                sb.recip(inv_s, s_ap)
                prev_out = out_t

    # Bacc.finalize runs the full compile pipeline (wait splitting into
    # event-semaphore NOPs, ACT table loads, InstISA byte codegen, ...).
    nc.finalize()
    return nc


def extract_params(S0, D0, sinc_w, rw_p, next_d_p, pls_p, sinc_p, best_sinc_p):
    return {
        "rw_p": [float(v) for v in np.asarray(rw_p, dtype=np.float32)],
        "next_d_p": [float(v) for v in np.asarray(next_d_p, dtype=np.float32)],
        "pls_p": [float(v) for v in np.asarray(pls_p, dtype=np.float32)],
        "sinc_p": [float(v) for v in np.asarray(sinc_p, dtype=np.float32)],
        "best_sinc_p": [float(v) for v in np.asarray(best_sinc_p, dtype=np.float32)],
        "sinc_w": [float(v) for v in np.asarray(sinc_w, dtype=np.float32)],
        "S0": [float(v) for v in np.asarray(S0, dtype=np.float32)],
        "D0": [float(v) for v in np.asarray(D0, dtype=np.float32)],
    }


def build_nc_cached(inputs_dict):
    d = dict(inputs_dict)
    d.pop("inputs", None)
    return build_bass(extract_params(**d))


def make_in_maps(inputs_dict):
    inputs = np.asarray(inputs_dict["inputs"], dtype=np.float32)
    dt_all = np.ascontiguousarray(inputs[:, :, 0])
    rat_all = np.ascontiguousarray(inputs[:, :, 1])
    lap_all = np.ascontiguousarray(inputs[:, :, 2])
    in_maps = []
    for c in range(NCORES):
        sl = slice(c * BC, (c + 1) * BC)
        in_maps.append({
            "dt": np.ascontiguousarray(dt_all[:, sl]).reshape(T, P, FD),
            "rat": np.ascontiguousarray(rat_all[:, sl]).reshape(T, P, FD),
            "lap": np.ascontiguousarray(lap_all[:, sl]).reshape(T, P, FD),
        })
    return in_maps


def kernel(inputs, S0, D0, sinc_w, rw_p, next_d_p, pls_p, sinc_p, best_sinc_p):
    from concourse.bass_utils import run_bass_kernel_spmd

    inputs = np.asarray(inputs, dtype=np.float32)
    params = {
        "rw_p": [float(v) for v in np.asarray(rw_p, dtype=np.float32)],
        "next_d_p": [float(v) for v in np.asarray(next_d_p, dtype=np.float32)],
        "pls_p": [float(v) for v in np.asarray(pls_p, dtype=np.float32)],
        "sinc_p": [float(v) for v in np.asarray(sinc_p, dtype=np.float32)],
        "best_sinc_p": [float(v) for v in np.asarray(best_sinc_p, dtype=np.float32)],
        "sinc_w": [float(v) for v in np.asarray(sinc_w, dtype=np.float32)],
        "S0": [float(v) for v in np.asarray(S0, dtype=np.float32)],
        "D0": [float(v) for v in np.asarray(D0, dtype=np.float32)],
    }

    nc = build_bass(params)

    dt_all = np.ascontiguousarray(inputs[:, :, 0])
    rat_all = np.ascontiguousarray(inputs[:, :, 1])
    lap_all = np.ascontiguousarray(inputs[:, :, 2])

    in_maps = []
    for c in range(NCORES):
        sl = slice(c * BC, (c + 1) * BC)
        in_maps.append({
            "dt": np.ascontiguousarray(dt_all[:, sl]).reshape(T, P, FD),
            "rat": np.ascontiguousarray(rat_all[:, sl]).reshape(T, P, FD),
            "lap": np.ascontiguousarray(lap_all[:, sl]).reshape(T, P, FD),
        })

    res = run_bass_kernel_spmd(nc, in_maps, core_ids=list(range(NCORES)))
    global LAST_RESULT
    LAST_RESULT = res

    outs = np.empty((T, B, 2), dtype=np.float32)
    for c in range(NCORES):
        sl = slice(c * BC, (c + 1) * BC)
        co = res.results[c]["out"].reshape(T, P, 2, FD)
        outs[:, sl, 0] = co[:, :, 0, :].reshape(T, BC)
        outs[:, sl, 1] = co[:, :, 1, :].reshape(T, BC)
    return outs, outs[-1].copy()
